# revision 1
# baseline (speedup 1.0000x reference)
"""GroupedQueryAttention Trainium2 Bass kernel — v4 (bf16, no collectives).

Sharding: 8 cores = data-parallel over batch (2) x strided row-sharding over
the sequence (4 shards).  Within batch b, core r in {0..3} owns the 128-row
tiles {4k + r : k in 0..3}.  Each core redundantly computes the full K/V,
projects Q for its own 512 rows, runs causal attention for those rows, and
applies the full output projection.  Host only gathers/scatters row slices —
no collective, no cross-core arithmetic.  The SPMD program is identical on
every core; per-core variation (rows, diagonal causal mask, rope phases)
comes in as input data.

Phase order is chosen for engine overlap: Q projection + Q-rope first (PE
warms up behind a single 2 MB DMA), then K/V per kv-head with K-rope emitted
immediately after each head so attention on early heads overlaps projection
of later ones.  V/Q/attention-output live in per-128-tile SBUF tiles so the
Tile scheduler sees fine-grained deps.  Scores for 4 s-tiles share one PSUM
bank -> one wide exp per group; the softmax denominator accumulates on the
PE via an all-ones bf16 stationary (fp32 in PSUM, broadcast across
partitions for free).
"""

import sys

for _p in ("/opt/trn_rl_repo",):
    if _p not in sys.path:
        sys.path.insert(0, _p)

import numpy as np

B, T, D = 2, 2048, 2048
NH, NKV, HD = 16, 4, 128
NREP = NH // NKV
P = 128
ND = D // P               # 16 contraction chunks
TCH = 512                 # t-chunk for the full-T projections
NTC = T // TCH            # 4
NSLOT = 4                 # row slots per core (128 rows each)
RW = NSLOT * P            # 512 own rows per core
KV = NKV * HD             # 512
NTT = T // P              # 16 sequence tiles
SCALE = float(HD) ** -0.5
THETA = 10000.0
NEG = -1.0e30
CORES = list(range(8))

_prog_cache = {}


def _build_program(reps=1):
    from contextlib import ExitStack
    from concourse import mybir, tile, bacc

    f32 = mybir.dt.float32
    bf16 = mybir.dt.bfloat16
    nc = bacc.Bacc("TRN2", target_bir_lowering=False, debug=False,
                   enable_asserts=True, num_devices=8)

    # host-prearranged inputs: every tile loads as one contiguous block
    xP = nc.dram_tensor("xP", [NTC, P, ND * TCH], bf16, kind="ExternalInput").ap()
    xoP = nc.dram_tensor("xoP", [P, ND * RW], bf16, kind="ExternalInput").ap()
    wqP = nc.dram_tensor("wqP", [NH, P, ND * P], bf16, kind="ExternalInput").ap()
    wkP = nc.dram_tensor("wkP", [P, ND * KV], bf16, kind="ExternalInput").ap()
    wvP = nc.dram_tensor("wvP", [P, ND * KV], bf16, kind="ExternalInput").ap()
    woP = nc.dram_tensor("woP", [4, P, ND * TCH], bf16, kind="ExternalInput").ap()
    ropeC = nc.dram_tensor("ropeC", [HD, T], bf16, kind="ExternalInput").ap()
    ropeS = nc.dram_tensor("ropeS", [HD, T], bf16, kind="ExternalInput").ap()
    ropeCq = nc.dram_tensor("ropeCq", [HD, 4 * RW], bf16, kind="ExternalInput").ap()
    ropeSq = nc.dram_tensor("ropeSq", [HD, 4 * RW], bf16, kind="ExternalInput").ap()
    maskw = nc.dram_tensor("maskw", [P, 16 * P], bf16, kind="ExternalInput").ap()
    out_ext = nc.dram_tensor("out", [RW, D], f32, kind="ExternalOutput").ap()

    Exp = mybir.ActivationFunctionType.Exp

    with tile.TileContext(nc) as tc, ExitStack() as es:
        perm = es.enter_context(tc.tile_pool(name="perm", bufs=1))
        kT = [perm.tile([P, T], bf16, tag=f"kT{g}", name=f"kT{g}")
              for g in range(NKV)]
        v_sb = [perm.tile([P, KV], bf16, tag=f"v{ti}", name=f"v{ti}")
                for ti in range(NTT)]
        qG = [perm.tile([P, 4 * RW], bf16, tag=f"qG{g}", name=f"qG{g}")
              for g in range(NKV)]
        aoG = [perm.tile([P, 4 * RW], bf16, tag=f"aoG{g}", name=f"aoG{g}")
               for g in range(NKV)]
        ones = perm.tile([P, P], bf16, tag="ones", name="ones")
        nc.vector.memset(ones, 1.0)

        for _rep in range(reps):
            # ---------------- P2: q projection + q rope ----------------
            with tc.tile_pool(name="p2w", bufs=3) as p2w, \
                 tc.tile_pool(name="p2x", bufs=1) as p2x, \
                 tc.tile_pool(name="p2c", bufs=1) as p2c, \
                 tc.tile_pool(name="p2s", bufs=2) as p2s, \
                 tc.tile_pool(name="psB", bufs=4, space="PSUM") as psB:
                xo = p2x.tile([P, ND * RW], bf16, tag="xo", name="xo")
                nc.sync.dma_start(out=xo, in_=xoP)
                rcq = p2c.tile([P, 4 * RW], bf16, tag="ropeCq", name="ropeCq")
                nc.gpsimd.dma_start(out=rcq, in_=ropeCq)
                rsq = p2c.tile([P, 4 * RW], bf16, tag="ropeSq", name="ropeSq")
                nc.gpsimd.dma_start(out=rsq, in_=ropeSq)
                for g in range(NKV):
                    for hh in range(NREP):
                        e = g * NREP + hh
                        wq_sb = p2w.tile([P, ND * P], bf16, tag="wq", name="wq")
                        nc.sync.dma_start(out=wq_sb, in_=wqP[e])
                        ps = psB.tile([P, RW], f32, tag="q", name="q")
                        for d in range(ND):
                            nc.tensor.matmul(
                                ps,
                                wq_sb[:, d * P:(d + 1) * P],
                                xo[:, d * RW:(d + 1) * RW],
                                start=(d == 0), stop=(d == ND - 1))
                        # scatter the 4 slot blocks into (k, hh, t) order
                        nc.scalar.copy(
                            out=qG[g].rearrange("p (k h t) -> p k h t",
                                                k=NSLOT, h=NREP)[:, :, hh, :],
                            in_=ps.rearrange("p (k t) -> p k t", k=NSLOT))
                    tl = qG[g]
                    sw = p2s.tile([P, 4 * RW], bf16, tag="swq", name="swq")
                    nc.sync.dma_start(out=sw[0:64, :], in_=tl[64:128, :])
                    nc.sync.dma_start(out=sw[64:128, :], in_=tl[0:64, :])
                    nc.vector.tensor_mul(sw, sw, rsq)
                    nc.vector.tensor_mul(tl, tl, rcq)
                    nc.vector.tensor_add(tl, tl, sw)

            # ---------------- P1: k/v projections + k rope ----------------
            with tc.tile_pool(name="p1w", bufs=1) as p1w, \
                 tc.tile_pool(name="p1x", bufs=1) as p1x, \
                 tc.tile_pool(name="p1c", bufs=1) as p1c, \
                 tc.tile_pool(name="p1s", bufs=2) as p1s, \
                 tc.tile_pool(name="psA", bufs=4, space="PSUM") as psA:
                wk_sb = p1w.tile([P, ND * KV], bf16, tag="wk", name="wk")
                nc.sync.dma_start(out=wk_sb, in_=wkP)
                wv_sb = p1w.tile([P, ND * KV], bf16, tag="wv", name="wv")
                nc.gpsimd.dma_start(out=wv_sb, in_=wvP)
                rc = p1c.tile([P, T], bf16, tag="ropeC", name="ropeC")
                nc.gpsimd.dma_start(out=rc, in_=ropeC)
                rs = p1c.tile([P, T], bf16, tag="ropeS", name="ropeS")
                nc.gpsimd.dma_start(out=rs, in_=ropeS)
                # all 4 x chunks stay resident for the whole phase
                xq = [p1x.tile([P, ND * TCH], bf16, tag=f"xq{tq}",
                               name=f"xq{tq}") for tq in range(NTC)]
                for tq in range(NTC):
                    nc.gpsimd.dma_start(out=xq[tq], in_=xP[tq])

                for g in range(NKV):
                    for tq in range(NTC):
                        ps = psA.tile([P, TCH], f32, tag="proj", name="proj")
                        for d in range(ND):
                            nc.tensor.matmul(
                                ps,
                                wk_sb[:, d * KV + g * HD:d * KV + (g + 1) * HD],
                                xq[tq][:, d * TCH:(d + 1) * TCH],
                                start=(d == 0), stop=(d == ND - 1))
                        nc.scalar.copy(
                            out=kT[g][:, tq * TCH:(tq + 1) * TCH], in_=ps)
                        if g == 0:
                            # v natural layout, per 128-row tile
                            for i in range(TCH // P):
                                ti = (TCH // P) * tq + i
                                psv = psA.tile([P, KV], f32, tag="proj",
                                               name="proj")
                                for d in range(ND):
                                    nc.tensor.matmul(
                                        psv,
                                        xq[tq][:, d * TCH + i * P:
                                           d * TCH + (i + 1) * P],
                                        wv_sb[:, d * KV:(d + 1) * KV],
                                        start=(d == 0), stop=(d == ND - 1))
                                nc.vector.tensor_copy(out=v_sb[ti], in_=psv)
                    # rope this kv head now so attention on it can start
                    sw = p1s.tile([P, T], bf16, tag="sw", name="sw")
                    nc.sync.dma_start(out=sw[0:64, :], in_=kT[g][64:128, :])
                    nc.sync.dma_start(out=sw[64:128, :], in_=kT[g][0:64, :])
                    nc.vector.tensor_mul(sw, sw, rs)
                    nc.vector.tensor_mul(kT[g], kT[g], rc)
                    nc.vector.tensor_add(kT[g], kT[g], sw)

            # ------------- P4 + P5: attention, then out-projection -------------
            with tc.tile_pool(name="amc", bufs=1) as amc, \
                 tc.tile_pool(name="aex", bufs=6) as aex, \
                 tc.tile_pool(name="aden", bufs=3) as aden, \
                 tc.tile_pool(name="p5w", bufs=1) as p5w, \
                 tc.tile_pool(name="p5y", bufs=3) as p5y, \
                 tc.tile_pool(name="psS", bufs=4, space="PSUM") as psS, \
                 tc.tile_pool(name="psO", bufs=2, space="PSUM") as psO, \
                 tc.tile_pool(name="psD", bufs=2, space="PSUM") as psD:
                mk = amc.tile([P, 16 * P], bf16, tag="maskw", name="maskw")
                nc.gpsimd.dma_start(out=mk, in_=maskw)
                # prefetch wo so the out-projection never waits on DMA
                wo_sb = [p5w.tile([P, ND * TCH], bf16, tag=f"wo{dg}",
                                  name=f"wo{dg}") for dg in range(4)]
                for dg in range(4):
                    nc.sync.dma_start(out=wo_sb[dg], in_=woP[dg])
                for g in range(NKV):
                    for k in range(NSLOT):
                        q_mv = qG[g][:, k * 4 * P:(k + 1) * 4 * P]
                        pso = psO.tile([P, 4 * P], f32, tag="av", name="av")
                        psd = psD.tile([P, 4 * P], f32, tag="db", name="db")
                        n_s = 4 * k + 4
                        for si in range(n_s):
                            psw = psS.tile([P, 4 * P], f32, tag="sc", name="sc")
                            nc.tensor.matmul(
                                psw, kT[g][:, si * P:(si + 1) * P], q_mv,
                                start=True, stop=True)
                            j = si - 4 * k
                            if j >= 0:
                                nc.vector.tensor_add(
                                    psw, psw, mk[:, j * 4 * P:(j + 1) * 4 * P])
                            exw = aex.tile([P, 4 * P], bf16, tag="exp", name="exp")
                            nc.scalar.activation(exw, psw, Exp, scale=SCALE)
                            nc.tensor.matmul(
                                psd, ones, exw,
                                start=(si == 0), stop=(si == n_s - 1),
                                skip_group_check=True)
                            nc.tensor.matmul(
                                pso,
                                v_sb[si][:, g * HD:(g + 1) * HD],
                                exw,
                                start=(si == 0), stop=(si == n_s - 1),
                                skip_group_check=True)
                        rden = aden.tile([P, 4 * P], f32, tag="rden", name="rden")
                        nc.vector.reciprocal(rden, psd)
                        nc.vector.tensor_mul(
                            aoG[g][:, k * 4 * P:(k + 1) * 4 * P], pso, rden)

                # out-projection (reuses the attention PSUM pools' banks)
                for k in range(NSLOT):
                    for dg in range(4):
                        psy = psS.tile([P, TCH], f32, tag="sc", name="y")
                        for h in range(NH):
                            g, hh = h // NREP, h % NREP
                            nc.tensor.matmul(
                                psy,
                                aoG[g][:, k * 4 * P + hh * P:
                                       k * 4 * P + (hh + 1) * P],
                                wo_sb[dg][:, h * TCH:(h + 1) * TCH],
                                start=(h == 0), stop=(h == NH - 1))
                        y_sb = p5y.tile([P, TCH], f32, tag="ysb", name="ysb")
                        nc.scalar.copy(out=y_sb, in_=psy)
                        nc.gpsimd.dma_start(
                            out=out_ext[k * P:(k + 1) * P,
                                        dg * TCH:(dg + 1) * TCH],
                            in_=y_sb)

    nc.compile()
    return nc


def _get_program(reps=1):
    if reps not in _prog_cache:
        _prog_cache[reps] = _build_program(reps)
    return _prog_cache[reps]


def _host_inputs(x, wq, wk, wv, wo):
    import ml_dtypes
    bf16 = ml_dtypes.bfloat16

    x = np.asarray(x, dtype=np.float32)
    wq = np.asarray(wq, dtype=np.float32)
    wk = np.asarray(wk, dtype=np.float32)
    wv = np.asarray(wv, dtype=np.float32)
    wo = np.asarray(wo, dtype=np.float32)

    # de-interleave head dims: even (real) components first, odd (imag) last
    perm128 = np.concatenate([np.arange(0, HD, 2), np.arange(1, HD, 2)])
    permq = np.concatenate([h * HD + perm128 for h in range(NH)])
    permk = np.concatenate([g * HD + perm128 for g in range(NKV)])

    wkP = np.ascontiguousarray(
        wk[permk].T.reshape(ND, P, KV).transpose(1, 0, 2).reshape(P, ND * KV)
        .astype(bf16))
    wvP = np.ascontiguousarray(
        wv.T.reshape(ND, P, KV).transpose(1, 0, 2).reshape(P, ND * KV)
        .astype(bf16))
    wqP = np.ascontiguousarray(
        wq[permq].T.reshape(ND, P, NH, P).transpose(2, 1, 0, 3)
        .reshape(NH, P, ND * P).astype(bf16))
    woP = np.ascontiguousarray(
        wo.T.reshape(ND, P, 4, TCH).transpose(2, 1, 0, 3)
        .reshape(4, P, ND * TCH).astype(bf16))

    freqs = (1.0 / THETA ** (np.arange(0, HD, 2)[: HD // 2] / HD)).astype(np.float64)
    t = np.arange(T, dtype=np.float64)
    ang = np.outer(freqs, t)                                  # (64, T)
    cos = np.cos(ang)
    sin = np.sin(ang)
    ropeC = np.concatenate([cos, cos], axis=0).astype(bf16)   # (128, T)
    ropeS = np.concatenate([-sin, sin], axis=0).astype(bf16)

    sp = np.arange(P)[:, None]
    tf = np.arange(P)[None, :]
    tri = np.where(sp <= tf, 0.0, NEG).astype(np.float32)     # (128, 128)

    in_maps = []
    for c in CORES:
        b, r = c // 4, c % 4
        rows = np.concatenate(
            [np.arange(128 * (4 * k + r), 128 * (4 * k + r) + P)
             for k in range(NSLOT)])
        xb = x[b].astype(bf16)
        xP = np.ascontiguousarray(
            xb.reshape(NTC, TCH, ND, P).transpose(0, 3, 2, 1)
            .reshape(NTC, P, ND * TCH))
        xo = xb[rows]                                         # (RW, D)
        xoP = np.ascontiguousarray(
            xo.reshape(RW, ND, P).transpose(2, 1, 0).reshape(P, ND * RW))
        maskw = np.concatenate(
            [np.tile(np.zeros((P, P), np.float32) if j < r
                     else tri if j == r
                     else np.full((P, P), NEG, np.float32), (1, 4))
             for j in range(4)], axis=1).astype(bf16)         # (128, 2048)
        rcq = ropeC[:, rows].reshape(HD, NSLOT, 1, P)
        rsq = ropeS[:, rows].reshape(HD, NSLOT, 1, P)
        rcqG = np.ascontiguousarray(
            np.broadcast_to(rcq, (HD, NSLOT, 4, P)).reshape(HD, 4 * RW))
        rsqG = np.ascontiguousarray(
            np.broadcast_to(rsq, (HD, NSLOT, 4, P)).reshape(HD, 4 * RW))
        in_maps.append({
            "xP": xP,
            "xoP": xoP,
            "wqP": wqP,
            "wkP": wkP,
            "wvP": wvP,
            "woP": woP,
            "ropeC": ropeC,
            "ropeS": ropeS,
            "ropeCq": rcqG,
            "ropeSq": rsqG,
            "maskw": maskw,
        })
    return in_maps


def _run(in_maps, reps=1):
    from concourse.bass_utils import run_bass_kernel_spmd
    nc = _get_program(reps)
    return run_bass_kernel_spmd(nc, in_maps, CORES)


def kernel(x, wq, wk, wv, wo, mask):
    import time
    in_maps = _host_inputs(x, wq, wk, wv, wo)
    try:
        res = _run(in_maps, reps=1)
    except Exception:
        # a previous heavy run can leave a core wedged
        # (NRT_EXEC_UNIT_UNRECOVERABLE); one retry recovers it
        time.sleep(2.0)
        res = _run(in_maps, reps=1)
    out = np.empty((B, T, D), dtype=np.float32)
    for c in CORES:
        b, r = c // 4, c % 4
        o = res.results[c]["out"].reshape(NSLOT, P, D)
        for k in range(NSLOT):
            out[b, 128 * (4 * k + r):128 * (4 * k + r) + P] = o[k]
    return out

